# revision 1
# baseline (speedup 1.0000x reference)
"""Dice loss (hard, argmax-based) for pred (2,8,128,128,128) f32, ref (2,1,128,128,128) i32.

Strategy: 8 NeuronCores, each handles one (batch, spatial-quarter) chunk of
S = 2^19 voxels. Per core, per F-wide tile:
  - one GpSimd (SWDGE) DMA loads the 8-channel pred slice, casting f32->bf16
    in the DMA (c-major layout: column c*F + f)
  - VectorE: bf16 max-tree over the 8 channels, then one broadcast is_ge
    (pred_c vs max, step-0 AP) in place over pred16 -> one-hot of argmax
    for all 8 channels (bf16 0/1)
  - VectorE+GpSimd: one-hot of ref labels (classes 1..7) into a
    block-interleaved tile (column tb*128 + c*16 + t); its class-0 slots are
    memset to 1.0 (a "ones" column set)
  - TensorE: for each 16-position block tb, matmul(stationary=ref-onehot
    block [single 128-col dim], moving=pred-onehot [c:8 x t:16 2D AP])
    accumulating a 128x128 PSUM.
PSUM decode (host): G[a,b] = sum_t M[a*16+t, b*16+t] with a = ref-side slot,
b = pred-side slot gives:
  G[c',c] = confusion count (argmax==c & ref==c') for c,c' in 1..7
  G[0, c] = psum_c  (count argmax==c, via ref-side ones)
  G[c',0] = rsum_c' (count ref==c', via pred-side ones)
which is exactly what the Dice formula needs.

bf16 rounding of pred can only create rare spurious argmax ties (~0.4% of
voxels); the effect on the final loss is ~1e-4 relative.
"""

import numpy as np

B = 2
C = 8
SPATIAL = 128 * 128 * 128  # 2097152 per (b, c)
N_CHUNKS = 4               # spatial quarters per batch element
S = SPATIAL // N_CHUNKS    # 524288 per core
P = 128
FTOT = S // P              # 4096
F = 1024                   # free-dim tile width per iteration
TBLK = 16                  # f-positions per matmul block

_CACHE = {}


def _build(
    S=S,
    F=F,
    bufs=2,
    ref_cast_act=False,
    bcast_eq=False,
    first_split=False,
    widths=None,
    ref_eq_pool=0,
):
    import concourse.mybir as mybir
    from concourse import bacc
    from concourse.tile import TileContext

    FTOT = S // P
    # list of (f0, width) tiles
    if widths is None:
        if first_split and FTOT // F >= 2:
            widths = [F // 2, F // 2] + [F] * (FTOT // F - 1)
        else:
            widths = [F] * (FTOT // F)
    assert sum(widths) == FTOT and all(w % TBLK == 0 for w in widths), widths
    offs = [sum(widths[:i]) for i in range(len(widths))]

    fp32 = mybir.dt.float32
    bf16 = mybir.dt.bfloat16
    i32 = mybir.dt.int32

    nc = bacc.Bacc()
    pred = nc.declare_dram_parameter("pred", [C, S], fp32, isOutput=False)
    ref = nc.declare_dram_parameter("ref", [S], i32, isOutput=False)
    conf = nc.declare_dram_parameter("conf", [P, P], fp32, isOutput=True)

    # (p, c, f) view of pred so the SBUF side of the DMA keeps partitions first
    predpcf = pred[:].rearrange("c (p f) -> p c f", p=P)
    ref2 = ref[:].rearrange("(p f) -> p f", p=P)

    with TileContext(nc) as tc:
        with (
            tc.tile_pool(name="pred16", bufs=bufs) as pred16_pool,
            tc.tile_pool(name="ohr", bufs=bufs) as ohr_pool,
            tc.tile_pool(name="mtree", bufs=2) as m_pool,
            tc.tile_pool(name="refp", bufs=2) as ref_pool,
            tc.tile_pool(name="outp", bufs=1) as out_pool,
            tc.tile_pool(name="psum", bufs=1, space="PSUM") as psum_pool,
        ):
            acc = psum_pool.tile([P, P], fp32)
            n_mm = sum(w // TBLK for w in widths)
            mm = 0
            for f0, Fi in zip(offs, widths):
                NBi = Fi // TBLK
                # ---- load pred slice, casting f32 -> bf16 in the DMA ----
                pred16 = pred16_pool.tile([P, C * Fi], bf16, tag="pred16")
                nc.gpsimd.dma_start(
                    out=pred16.rearrange("p (c f) -> p c f", c=C),
                    in_=predpcf[:, :, f0 : f0 + Fi],
                )

                # ---- ref labels ----
                ref32 = ref_pool.tile([P, Fi], i32, tag="ref32")
                nc.sync.dma_start(out=ref32[:], in_=ref2[:, f0 : f0 + Fi])
                ref16 = ref_pool.tile([P, Fi], bf16, tag="ref16")
                if ref_cast_act:
                    nc.scalar.copy(out=ref16[:], in_=ref32[:])
                else:
                    nc.vector.tensor_copy(out=ref16[:], in_=ref32[:])
                r16v = ref16.rearrange("p (nb t) -> p nb t", t=TBLK)

                # ---- max over channels (bf16 tensor_tensor, 2x mode) ----
                m1 = m_pool.tile([P, 4 * Fi], bf16, tag="m1")
                nc.vector.tensor_max(
                    out=m1[:], in0=pred16[:, : 4 * Fi], in1=pred16[:, 4 * Fi :]
                )
                m2 = m_pool.tile([P, 2 * Fi], bf16, tag="m2")
                nc.vector.tensor_max(
                    out=m2[:], in0=m1[:, : 2 * Fi], in1=m1[:, 2 * Fi :]
                )
                m3 = m_pool.tile([P, Fi], bf16, tag="m3")
                nc.vector.tensor_max(out=m3[:], in0=m2[:, :Fi], in1=m2[:, Fi:])

                # ---- one-hot argmax (all 8 channels), in place over pred16 ----
                # Keeping channel 0's one-hot (instead of a ones block) makes
                # the rsum margin tie-inflated consistently with inter/psum,
                # so the bf16 spurious-tie error cancels in the Dice ratio.
                if bcast_eq:
                    ppv = pred16.rearrange("p (c f) -> p c f", c=C)
                    nc.vector.tensor_tensor(
                        out=ppv,
                        in0=ppv,
                        in1=m3[:]
                        .rearrange("p (o f) -> p o f", o=1)
                        .broadcast_to([P, C, Fi]),
                        op=mybir.AluOpType.is_ge,
                    )
                else:
                    for c in range(C):
                        nc.vector.tensor_tensor(
                            out=pred16[:, c * Fi : (c + 1) * Fi],
                            in0=pred16[:, c * Fi : (c + 1) * Fi],
                            in1=m3[:],
                            op=mybir.AluOpType.is_ge,
                        )

                # ---- one-hot ref, block-interleaved: column tb*128 + c*16 + t
                ohr = ohr_pool.tile([P, NBi, C * TBLK], bf16, tag="ohr")
                r4 = ohr.rearrange("p nb (c t) -> p nb c t", t=TBLK)
                for c in range(1, C):
                    eng = nc.gpsimd if c <= ref_eq_pool else nc.vector
                    eng.tensor_scalar(
                        out=r4[:, :, c, :],
                        in0=r16v[:],
                        scalar1=float(c),
                        scalar2=None,
                        op0=mybir.AluOpType.is_equal,
                    )
                nc.gpsimd.memset(r4[:, :, 0, :], 1.0)

                # ---- confusion-matrix matmuls ----
                # stationary = ohr block (single free dim, 128 cols = (c_ref, t))
                # moving = pred16 one-hot 2D-free AP (c_pred:8 x t:16)
                p3 = pred16.rearrange("p (c f) -> p c f", c=C)
                for tb in range(NBi):
                    sl = slice(tb * TBLK, (tb + 1) * TBLK)
                    nc.tensor.matmul(
                        acc[:],
                        ohr[:, tb, :],
                        p3[:, :, sl],
                        start=(mm == 0),
                        stop=(mm == n_mm - 1),
                    )
                    mm += 1

            outt = out_pool.tile([P, P], fp32)
            nc.vector.tensor_copy(out=outt[:], in_=acc[:])
            nc.sync.dma_start(out=conf[:], in_=outt[:])

    nc.compile()
    return nc


BEST = dict(
    bufs=4,
    ref_cast_act=True,
    bcast_eq=True,
    widths=[640, 640, 640, 640, 640, 448, 448],
    ref_eq_pool=4,
)


def _get_nc():
    if "nc" not in _CACHE:
        _CACHE["nc"] = _build(**BEST)
    return _CACHE["nc"]


def _dice_from_margins(G):
    """G[a, b]: a = ref-side slot (0=ones), b = pred-side slot (argmax
    one-hot, incl. class 0). Mirrors reference(). rsum uses row sums over
    the pred one-hots so any bf16 argmax-tie inflation cancels between
    inter/psum/rsum in the Dice ratio."""
    G = G.astype(np.float32)
    inter = np.diag(G)[1:]
    psum = G[0, 1:]
    rsum = G[1:, :].sum(axis=1)
    hasref = rsum > 0
    union = psum + rsum
    dice = np.where(
        hasref, 2.0 * inter / np.where(union > 0, union, np.float32(1.0)), 0.0
    ).astype(np.float32)
    sumweights = hasref.astype(np.float32).sum()
    return dice.sum() / sumweights


def _make_in_maps(pred, ref):
    predr = pred.reshape(B, C, N_CHUNKS, S)
    refr = ref.reshape(B, N_CHUNKS, S)
    in_maps = []
    for k in range(B * N_CHUNKS):
        b, j = divmod(k, N_CHUNKS)
        in_maps.append(
            {
                "pred": np.ascontiguousarray(predr[b, :, j]),
                "ref": np.ascontiguousarray(refr[b, j]),
            }
        )
    return in_maps


def _get_executor():
    """Build (once) a cached jitted SPMD executor mirroring
    bass2jax.run_bass_via_pjrt, so repeated kernel() calls skip re-tracing
    and NEFF recompilation."""
    if "exec" in _CACHE:
        return _CACHE["exec"]

    import jax
    import jax.numpy as jnp  # noqa: F401
    from jax.sharding import Mesh, PartitionSpec
    from jax.experimental.shard_map import shard_map
    import concourse.mybir as mybir
    from concourse import bass2jax

    bass2jax.install_neuronx_cc_hook()
    nc = _get_nc()
    n_cores = B * N_CHUNKS

    partition_name = nc.partition_id_tensor.name if nc.partition_id_tensor else None

    in_names, out_names, out_avals, zero_shapes = [], [], [], []
    for alloc in nc.m.functions[0].allocations:
        if not isinstance(alloc, mybir.MemoryLocationSet):
            continue
        name = alloc.memorylocations[0].name
        if alloc.kind == "ExternalInput":
            if name != partition_name:
                in_names.append(name)
        elif alloc.kind == "ExternalOutput":
            shape = tuple(alloc.tensor_shape)
            dtype = mybir.dt.np(alloc.dtype)
            out_names.append(name)
            out_avals.append(jax.core.ShapedArray(shape, dtype))
            zero_shapes.append((shape, dtype))
    n_params = len(in_names)
    all_names = in_names + out_names
    if partition_name is not None:
        all_names = all_names + [partition_name]

    def _body(*args):
        operands = list(args)
        if partition_name is not None:
            operands.append(bass2jax.partition_id_tensor())
        outs = bass2jax._bass_exec_p.bind(
            *operands,
            out_avals=tuple(out_avals),
            in_names=tuple(all_names),
            out_names=tuple(out_names),
            lowering_input_output_aliases=(),
            sim_require_finite=True,
            sim_require_nnan=True,
            nc=nc,
        )
        return tuple(outs)

    devices = jax.devices()[:n_cores]
    mesh = Mesh(np.asarray(devices), ("core",))
    n_outs = len(out_names)
    sharded = jax.jit(
        shard_map(
            _body,
            mesh=mesh,
            in_specs=(PartitionSpec("core"),) * (n_params + n_outs),
            out_specs=(PartitionSpec("core"),) * n_outs,
            check_rep=False,
        ),
        donate_argnums=tuple(range(n_params, n_params + n_outs)),
        keep_unused=True,
    )
    _CACHE["exec"] = (sharded, in_names, out_names, out_avals, zero_shapes, n_cores)
    return _CACHE["exec"]


def _execute(in_maps):
    sharded, in_names, out_names, out_avals, zero_shapes, n_cores = _get_executor()
    concat_in = [
        np.concatenate([in_maps[c][nm] for c in range(n_cores)], axis=0)
        for nm in in_names
    ]
    concat_zeros = [
        np.zeros((n_cores * s[0], *s[1:]), dt) for (s, dt) in zero_shapes
    ]
    out_arrs = sharded(*concat_in, *concat_zeros)
    return [
        {
            nm: np.asarray(out_arrs[i]).reshape(n_cores, *out_avals[i].shape)[c]
            for i, nm in enumerate(out_names)
        }
        for c in range(n_cores)
    ]


def _decode(results):
    loss = np.float32(0.0)
    for b in range(B):
        G = np.zeros((C, C), dtype=np.float64)
        for j in range(N_CHUNKS):
            M = results[b * N_CHUNKS + j]["conf"].reshape(C, TBLK, C, TBLK)
            G += np.einsum("atbt->ab", M)
        loss += _dice_from_margins(G)
    return np.array(loss / np.float32(B), dtype=np.float32)


def run(pred, ref, trace=False, trace_cores=None):
    pred = np.asarray(pred, dtype=np.float32)
    ref = np.asarray(ref, dtype=np.int32)
    assert pred.shape == (B, C, 128, 128, 128), pred.shape
    assert ref.shape == (B, 1, 128, 128, 128), ref.shape

    in_maps = _make_in_maps(pred, ref)

    if trace:
        from concourse.bass_utils import run_bass_kernel_spmd

        res = run_bass_kernel_spmd(
            _get_nc(),
            in_maps,
            core_ids=list(range(B * N_CHUNKS)),
            trace=True,
            **({"trace_cores": trace_cores} if trace_cores is not None else {}),
        )
        return _decode(res.results), res

    try:
        results = _execute(in_maps)
    except Exception:
        from concourse.bass_utils import run_bass_kernel_spmd

        results = run_bass_kernel_spmd(
            _get_nc(), in_maps, core_ids=list(range(B * N_CHUNKS))
        ).results
    return _decode(results), None


def kernel(pred, ref):
    out, _ = run(pred, ref)
    return out

